# revision 17
# baseline (speedup 1.0000x reference)
"""Trainium2 Bass kernel for nn_Attention_layer_12249246728743.

Reference structure (after untangling the C-order reshape): per channel c
of 512, the 3136 raster positions split into 49 segments of 64
consecutive positions; each segment attends over a 7x7 shifted window of
its OWN channel plane (depthwise local attention):

  scores[c,s,p=(i,j)] = sum_d q[c,64s+d] * k[c, win(64s+d, i, j)]
                        + (sum_d q[c,64s+d]) * bias49[p]
  w = softmax_p(scores);  out[c,64s+d] = sum_p w[c,s,p] * v[c, win(...)]

with q/k/v = 1x1 convs of x (k, v on the zero-padded 62x62 domain).

Sharding: channel-parallel across 8 cores (64 channels each); every
attention segment is core-local: no halo, no collectives.

Layout: "pair-packed" attention - partitions = 64 channels x 2
shift-pair halves, free dim = the full 3136-position raster. The B half
holds k/v planes pre-shifted by +1 (or +56 for row-wrapping pairs), so
ONE tensor op computes two of the 49 window shifts at once. Pairs whose
window offsets form an arithmetic run (stride 2 within a row of the 7x7
table) are GROUPED: one 4D strided access pattern computes 3-4 pairs per
tensor op, and the segment-sum bf16 add trees run per group for levels
1-3; level-3 outputs land in shared slot tiles so levels 4/5 run once
across all tree pairs (amortizing per-op overhead). A few pairs offload
their reduction to the PE: stage 1 is 8 strided-rhs identity matmuls
into PSUM 8-sums per pair; stage 2 reduces the first offloaded pairs in
8 shared matmuls while the last two reduce on a vector engine (finest
PE<->vector balance knob). Scores are stored bf16 (the
add trees already round every level to bf16; one more rounding is
noise). The softmax runs WITHOUT a
device max pass: the positional-bias part of the score (qsum x b49,
host-known) is shifted host-side so device scores are bounded above by
~+4; large negatives underflow exp to an exact 0, which softmax
tolerates. AV weights enter the multiply as stride-0 broadcast APs, and
the AV product tiles are accumulated on the PE via half-folding matmuls
(lhsT F[p, p%64]=1) that also combine the two shift halves, so the
output lands directly as [64, 3136] in PSUM. x and the conv weights ship
as bf16 pairs packed in fp32 words (halves the load DMA); the conv runs
in bf16 with fp32 PSUM accumulation. Work is split between DVE and
GPSIMD by an exact subset-sum makespan balancer.
"""

import numpy as np

import concourse.bass as bass
import concourse.mybir as mybir
import concourse.tile as tile
from concourse.bass_utils import run_bass_kernel_spmd
from bass_rust import AP

F32 = mybir.dt.float32
BF16 = mybir.dt.bfloat16
AX = mybir.AxisListType
OP = mybir.AluOpType
AF = mybir.ActivationFunctionType

N_CORES = 8
C = 512
H = W = 56
HP = WP = 62          # padded spatial
NPOS = H * W          # 3136
NPAD = HP * WP        # 3844
K = 7
NSH = K * K           # 49 shifts
SEG = 64              # positions per attention segment
NSEG = NPOS // SEG    # 49 segments per channel
CH = 64               # channels per core
NPAIR = 25            # 24 shift pairs + 1 single (p=48)

# shift pairs (pA=2t, pB=2t+1): B half of the k/v tiles is pre-shifted by
# +1 (same-row j->j+1) or +56 (row wrap (i,6)->(i+1,0)). Window slice is
# always pA's (i,j) clamped to j<=6.
def _pair_table():
    pairs = []
    for t in range(24):
        pA = 2 * t
        i, j = divmod(pA, K)
        if j < K - 1:
            pairs.append((i, j, "k1"))     # B = (i, j+1) via +1 tile
        else:
            pairs.append((i, j, "k56"))    # B = (i+1, 0) via +56 tile
    pairs.append((6, 6, "k1"))             # p=48 single; B half masked
    return pairs


PAIRS = _pair_table()

# groups of pairs whose A-shift offsets are an arithmetic run (stride 2,
# same row i, same source plane): (q0, G, i, j0, plane). One 4D AP
# computes the whole group's products.
GROUPS = [
    (0, 3, 0, 0, "k1"),
    (3, 1, 0, 6, "k56"),
    (4, 3, 1, 1, "k1"),
    (7, 3, 2, 0, "k1"),
    (10, 1, 2, 6, "k56"),
    (11, 3, 3, 1, "k1"),
    (14, 3, 4, 0, "k1"),
    (17, 1, 4, 6, "k56"),
    (18, 3, 5, 1, "k1"),
    (21, 4, 6, 0, "k1"),
]
# PE offload: (group index, #pairs from the group's tail). Tuned so the
# three-engine makespan balances.
OFFLOAD = [(1, 1), (4, 1), (7, 1), (9, 2)]

# --- naive op-cost mirror (documented TRN2 formulas) for the balancer ---
def _c_tt(fd, bf16_out=True):
    return (58 + (fd / 2 if bf16_out else fd)) / 0.96


class _Sched:
    """DVE/GPSIMD makespan balancer (Pool runs at 2x cost).

    Costs are static, so assignment is planned offline: exact subset-sum
    DP over the schedulable costs against the DVE-pinned base load,
    consumed in emission order by pick().
    """

    def __init__(self, nc, pe_base):
        self.nc = nc
        self.v = 0.0
        self.g = 0.0
        self.pe = pe_base
        self.plan = []

    def make_plan(self, costs, pinned_v, pinned_g=0.0):
        # exact subset-sum DP: choose the GPSIMD subset G minimizing
        # max(pinned_v + S - G, pinned_g + 2G) — globally optimal split
        ic = [int(round(c)) for c in costs]
        total = sum(ic)
        reach = {0: None}
        for idx, c in enumerate(ic):
            new = {}
            for s, _ in reach.items():
                t = s + c
                if t not in reach and t not in new:
                    new[t] = (s, idx)
            reach.update(new)
        pv = int(round(pinned_v))
        pg = int(round(pinned_g))
        best_g = min(reach, key=lambda G: max(pv + total - G, pg + 2 * G))
        gset = set()
        s = best_g
        while s:
            prev, idx = reach[s]
            gset.add(idx)
            s = prev
        asg = ["g" if i in gset else "v" for i in range(len(ic))]
        self.plan = asg[::-1]          # consume via pop()

    def pick(self, cost):
        e = self.plan.pop() if self.plan else (
            "v" if max(self.v + cost, self.g) <= max(self.v,
                                                     self.g + 2 * cost)
            else "g")
        if e == "v":
            self.v += cost
            return self.nc.vector
        self.g += 2 * cost
        return self.nc.gpsimd



def _build_nc():
    nc = bass.Bass()

    # x / conv weights ship as bf16 pairs packed in fp32 words
    xpb = nc.declare_dram_parameter("xpb", [128, 2 * NPAD], F32,
                                    isOutput=False)
    wTb = nc.declare_dram_parameter("wTb", [128, 4 * 96], F32,
                                    isOutput=False)
    bqk = nc.declare_dram_parameter("bqk", [2 * CH, 1], F32, isOutput=False)
    bv = nc.declare_dram_parameter("bv", [CH, 1], F32, isOutput=False)
    bk0 = nc.declare_dram_parameter("bk0", [CH, 1], F32, isOutput=False)
    biasS = nc.declare_dram_parameter("biasS", [128, NPAIR * NSEG], F32,
                                      isOutput=False)
    identp = nc.declare_dram_parameter("identp", [128, 64], F32,
                                       isOutput=False)
    foldp = nc.declare_dram_parameter("foldp", [128, 32], F32,
                                      isOutput=False)
    out_d = nc.declare_dram_parameter("out", [CH, NPOS], F32, isOutput=True)

    RPC = 8                      # rows per conv chunk
    NCHI = RPC * W               # 448 interior positions per chunk
    NCK = 7                      # conv chunks (7*8 = 56 rows)
    AVC = 448                    # AV psum chunk (7 segments)
    NAV = 7                      # AV chunks
    NQ = NPAIR * NSEG            # 1225
    NOFF = sum(n for _, n in OFFLOAD)   # PE-offloaded pair count

    with tile.TileContext(nc) as tc:
        with (
            tc.tile_pool(name="persist", bufs=1) as pp,
            tc.tile_pool(name="work", bufs=2) as wp,
            tc.tile_pool(name="psum", bufs=1, space="PSUM") as psp,
        ):
            MM = lambda n: max(60, 6 + n) / 2.4 + 128 / 1.2
            PE_BASE = (56 * MM(NCHI) + 25 * 7 * MM(448)
                       + NOFF * 8 * MM(392) + 8 * MM((NOFF - 2) * NSEG))
            sch = _Sched(nc, PE_BASE)

            # ---- loads ----
            xt_all = pp.tile([128, 2 * NPAD], F32, tag="xall", name="xall")
            wt_all = pp.tile([128, 4 * 96], F32, tag="wall", name="wall")
            for s0 in range(0, 2 * NPAD, NPAD):
                nc.sync.dma_start(xt_all[:, s0:s0 + NPAD],
                                  xpb[:, s0:s0 + NPAD])
            nc.sync.dma_start(wt_all[:], wTb[:])
            xtb = xt_all[:].bitcast(BF16).rearrange("p (k n) -> p k n", k=4)
            xt = [xtb[:, kt, :] for kt in range(4)]
            wtb = wt_all[:].bitcast(BF16).rearrange("p (k n) -> p k n", k=4)
            wt = [wtb[:, kt, :] for kt in range(4)]
            bqk_s = pp.tile([128, 1], F32, tag="bqk", name="bqk")
            bv_s = pp.tile([CH, 1], F32, tag="bv", name="bv")
            bk0_s = pp.tile([CH, 1], F32, tag="bk0", name="bk0")
            nc.sync.dma_start(bk0_s[:], bk0[:])
            bS_s = pp.tile([128, NQ], F32, tag="bS", name="bS")
            idf_s = pp.tile([128, 64], F32, tag="idf", name="idf")
            fld_s = pp.tile([128, 32], F32, tag="fld", name="fld")
            nc.sync.dma_start(bqk_s[:], bqk[:])
            nc.sync.dma_start(bv_s[:], bv[:])
            nc.sync.dma_start(bS_s[:], biasS[:])
            nc.sync.dma_start(idf_s[:], identp[:])
            nc.sync.dma_start(fld_s[:], foldp[:])
            ident = idf_s[:].bitcast(BF16)          # [128, 128] bf16 identity
            fold = fld_s[:].bitcast(BF16)           # [128, 64] bf16 half-fold

            # ---- attention-layout staging tiles ----
            qb = pp.tile([128, NPOS], BF16, tag="qb", name="qb")
            k1 = pp.tile([128, NPAD], BF16, tag="k1", name="k1")
            k56 = pp.tile([128, NPAD], BF16, tag="k56", name="k56")
            v1 = pp.tile([128, NPAD], BF16, tag="v1", name="v1")
            v56 = pp.tile([128, NPAD], BF16, tag="v56", name="v56")

            # conv / QK-tree / AV shared PSUM accumulators (8 banks)
            psa = [psp.tile([128, 512], F32, tag=f"psa{j}", name=f"psa{j}")
                   for j in range(8)]

            # ---- 1x1 convs on interior rows (rows 3..58 of padded);
            # rhs is a 3D interior view [128, 8, 56] (448 cols) in bf16 --
            for ci in range(NCK):
                r0 = 3 + RPC * ci
                pq = psa[(2 * ci) % 6]
                pv = psa[(2 * ci) % 6 + 1]
                for kt in range(4):
                    x3 = xt[kt].rearrange("a (r c) -> a r c", c=WP)
                    rhs = x3[:, r0:r0 + RPC, 3:59]
                    nc.tensor.matmul(pq[:, 0:NCHI], lhsT=wt[kt][:, 0:128],
                                     rhs=rhs, start=(kt == 0), stop=(kt == 3))
                    nc.tensor.matmul(pv[0:CH, 0:NCHI],
                                     lhsT=wt[kt][:, 128:192],
                                     rhs=rhs, start=(kt == 0), stop=(kt == 3))
                pq3 = pq[:, 0:NCHI].rearrange("a (r c) -> a r c", c=W)
                pv3 = pv[0:CH, 0:NCHI].rearrange("a (r c) -> a r c", c=W)
                qbv = qb[:, NCHI * ci: NCHI * (ci + 1)].rearrange(
                    "a (r c) -> a r c", c=W)
                nc.scalar.activation(qbv[0:CH], pq3[0:CH],
                                     AF.Identity, bias=bqk_s[0:CH, :])
                nc.scalar.activation(qbv[CH:128], pq3[0:CH],
                                     AF.Identity, bias=bqk_s[0:CH, :])
                k13 = k1[0:CH, :].rearrange("a (r c) -> a r c", c=WP)
                v13 = v1[0:CH, :].rearrange("a (r c) -> a r c", c=WP)
                nc.scalar.activation(k13[:, r0:r0 + RPC, 3:59],
                                     pq3[CH:128], AF.Identity,
                                     bias=bqk_s[CH:128, :])
                nc.scalar.activation(v13[:, r0:r0 + RPC, 3:59],
                                     pv3, AF.Identity,
                                     bias=bv_s[:])

            # ---- k/v padded borders hold conv(0)+bias = bias ----
            zt = pp.tile([CH, 1], BF16, tag="zt", name="zt")
            nc.vector.memset(zt[:], 0.0)
            sch.v += (58 + 1) / 0.96
            for plane, bias in ((k1, bk0_s[:]), (v1, bv_s[:])):
                p3 = plane[0:CH, :].rearrange("a (r c) -> a r c", c=WP)
                for view in (
                    plane[0:CH, 0: 3 * WP],                # rows 0-2
                    plane[0:CH, 59 * WP: NPAD],            # rows 59-61
                    p3[:, 3:59, 0:3],                      # left cols
                    p3[:, 3:59, 59:62],                    # right cols
                ):
                    sh = view.shape
                    if len(sh) == 2:
                        zin = zt[:, 0:1].broadcast_to((CH, sh[1]))
                    else:
                        zin = zt[:, 0:1].rearrange(
                            "a (x y) -> a x y", y=1).broadcast_to(
                            (CH, sh[1], sh[2]))
                    nc.scalar.activation(view, zin, AF.Identity, bias=bias)

            # ---- stage shifted B halves (+1 / +56) and k56/v56 A halves --
            for src, d1, d56 in ((k1, k1, k56), (v1, v1, v56)):
                nc.scalar.copy(d1[CH:128, 0:NPAD - 1], src[0:CH, 1:NPAD])
                nc.scalar.copy(d56[0:CH, :], src[0:CH, :])
                nc.scalar.copy(d56[CH:128, 0:NPAD - 56], src[0:CH, 56:NPAD])
                nc.scalar.copy(d1[CH:128, NPAD - 1: NPAD],
                               zt[:, 0:1].broadcast_to((CH, 1)))
                nc.scalar.copy(d56[CH:128, NPAD - 56: NPAD],
                               zt[:, 0:1].broadcast_to((CH, 56)))

            # ---- QK: grouped pair products + bf16 add trees -> scores ----
            S_all = pp.tile([128, NQ], BF16, tag="Sall", name="Sall")
            qb3 = qb[:].rearrange("a (x y) -> a x y", y=W)
            t8_all = pp.tile([128, NOFF * NSEG * 8], BF16, tag="t8a",
                             name="t8a")

            def gwin(t, i, j0, G):
                # [128, G, 56, 56] overlapping window run, stride 2 in j
                base = t[:]
                pstride = base.ap[0][0]
                return AP(base.tensor, WP * i + j0,
                          [[pstride, 128], [2, G], [WP, H], [1, W]])

            offload = {}                 # group idx -> n tail pairs on PE
            for gi, n in OFFLOAD:
                offload[gi] = n

            # tree-chain slot layout: chains write their level-3 outputs
            # into shared slot-tiles so levels 4/5 run ONCE across all
            # tree pairs (amortizes the 58-cycle op overhead); the last 2
            # offloaded pairs' stage-2 runs as a V tree instead of PE.
            NTT = sum(G - offload.get(g, 0)
                      for g, (_, G, _, _, _) in enumerate(GROUPS))
            NPE2 = NOFF - 2              # pairs reduced by PE stage 2
            # balancer cost table (emission order); tree levels are
            # separate items so the subset-sum split is fine-grained
            COSTS = []
            chains = []
            so = 0
            for g, (q0, G, i, j0, pl) in enumerate(GROUPS):
                COSTS.append(_c_tt(G * NPOS))          # product
                ntree = G - offload.get(g, 0)
                if ntree:
                    COSTS += [_c_tt(ntree * NSEG * w) for w in (32, 16, 8)]
                    chains.append((so, q0, ntree))
                    so += ntree
            COSTS += [_c_tt(NTT * NSEG * 4)]           # global level 4
            COSTS += [_c_tt(NTT * NSEG * 2)]           # global level 5
            COSTS += [_c_tt(n * NSEG) for _, _, n in chains]   # finals
            S2V = (_c_tt(2 * NSEG * 4) + _c_tt(2 * NSEG * 2)
                   + _c_tt(2 * NSEG))
            COSTS += [S2V]                             # stage-2-on-V pairs
            MTREE = (_c_tt(12 * NSEG) + _c_tt(6 * NSEG) + _c_tt(3 * NSEG)
                     + 3 * _c_tt(NSEG))
            COSTS += [_c_tt(NQ, False)]                # S_b bias add
            COSTS += [MTREE]                           # E denominator tree
            COSTS += [_c_tt(NQ)]                       # EN
            COSTS += [_c_tt(G * NPOS) for (_, G, _, _, _) in GROUPS]  # AV
            PINNED_V = ((58 + 1) / 0.96
                        + (58 + NSEG) / 0.96           # den
                        + (58 + NSEG) / 0.96)          # reciprocal
            # Bass-init emits 4 const-AP memsets on GPSIMD
            PINNED_G = 4 * 2 * (58 + 1) / 0.96
            sch.make_plan(COSTS, PINNED_V, PINNED_G)
            t8a = pp.tile([128, NTT * NSEG * 8], BF16, tag="t8a2",
                          name="t8a2")
            t4a = pp.tile([128, NTT * NSEG * 4], BF16, tag="t4a",
                          name="t4a")
            t2a = pp.tile([128, NTT * NSEG * 2], BF16, tag="t2a",
                          name="t2a")

            flip = 0
            toff = 0                     # filled t8_all pair slots
            off_runs = []                # (t8 slot start, q start, n)
            for g, (q0, G, i, j0, pl) in enumerate(GROUPS):
                ktile = k1 if pl == "k1" else k56
                gp = wp.tile([128, 4 * NPOS], BF16, tag="gp", name="gp",
                             bufs=2)
                eng = sch.pick(_c_tt(G * NPOS))
                eng.tensor_tensor(
                    out=gp[:, 0:G * NPOS].rearrange(
                        "a (g x y) -> a g x y", g=G, y=W),
                    in0=gwin(ktile, i, j0, G),
                    in1=qb3.unsqueeze(1).broadcast_to((128, G, H, W)),
                    op=OP.mult)
                noff = offload.get(g, 0)
                ntree = G - noff
                # PE stage 1 for the group's tail pairs
                for t in range(noff):
                    p3 = gp[:, (ntree + t) * NPOS:
                            (ntree + t + 1) * NPOS].rearrange(
                        "a (s d) -> a s d", d=SEG)
                    pt = psa[flip]
                    flip ^= 1
                    for off in range(0, SEG, 8):
                        nc.tensor.matmul(
                            pt[:, 0:NSEG * 8], lhsT=ident,
                            rhs=p3[:, :, off:off + 8],
                            start=(off == 0), stop=(off == SEG - 8))
                    nc.scalar.copy(
                        t8_all[:, (toff + t) * NSEG * 8:
                               (toff + t + 1) * NSEG * 8].rearrange(
                            "a (s d) -> a s d", d=8),
                        pt[:, 0:NSEG * 8].rearrange("a (s d) -> a s d",
                                                    d=8))
                if noff:
                    off_runs.append((toff, q0 + ntree, noff))
                    toff += noff
                # DVE tree levels 1-3 over the group's head pairs; level 3
                # lands in the shared slot tile for the global levels
                if ntree:
                    cur = gp[:, 0:ntree * NPOS]
                    slot = [c for c in chains if c[1] == q0][0][0]
                    for w in (32, 16):
                        t = wp.tile([128, 3 * NSEG * w], BF16,
                                    tag=f"t{w}", name=f"t{w}",
                                    bufs=1 if w == 32 else 2)
                        c3 = cur.rearrange("a (s d) -> a s d", d=2 * w)
                        t_o = t[:, 0:ntree * NSEG * w]
                        eng = sch.pick(_c_tt(ntree * NSEG * w))
                        eng.tensor_tensor(
                            out=t_o.rearrange("a (s d) -> a s d", d=w),
                            in0=c3[:, :, 0:w], in1=c3[:, :, w:2 * w],
                            op=OP.add)
                        cur = t_o
                    c3 = cur.rearrange("a (s d) -> a s d", d=16)
                    eng = sch.pick(_c_tt(ntree * NSEG * 8))
                    eng.tensor_tensor(
                        out=t8a[:, slot * NSEG * 8:
                                (slot + ntree) * NSEG * 8].rearrange(
                            "a (s d) -> a s d", d=8),
                        in0=c3[:, :, 0:8], in1=c3[:, :, 8:16], op=OP.add)

            # global tree levels 4/5 across all tree pairs' slots
            c3 = t8a[:].rearrange("a (s d) -> a s d", d=8)
            eng = sch.pick(_c_tt(NTT * NSEG * 4))
            eng.tensor_tensor(
                out=t4a[:].rearrange("a (s d) -> a s d", d=4),
                in0=c3[:, :, 0:4], in1=c3[:, :, 4:8], op=OP.add)
            c3 = t4a[:].rearrange("a (s d) -> a s d", d=4)
            eng = sch.pick(_c_tt(NTT * NSEG * 2))
            eng.tensor_tensor(
                out=t2a[:].rearrange("a (s d) -> a s d", d=2),
                in0=c3[:, :, 0:2], in1=c3[:, :, 2:4], op=OP.add)
            for slot, q0c, n in chains:
                c3 = t2a[:, slot * NSEG * 2:
                         (slot + n) * NSEG * 2].rearrange(
                    "a (s d) -> a s d", d=2)
                eng = sch.pick(_c_tt(n * NSEG))
                eng.tensor_tensor(
                    out=S_all[:, NSEG * q0c: NSEG * (q0c + n)].rearrange(
                        "a (s o) -> a s o", o=1),
                    in0=c3[:, :, 0:1], in1=c3[:, :, 1:2], op=OP.add)

            # PE stage 2 for the first NPE2 offloaded pairs' 8-sums,
            # 8 shared strided-rhs matmuls into one PSUM bank.
            t84 = t8_all[:].rearrange("a (p s d) -> a p s d", s=NSEG, d=8)
            for off in range(8):
                nc.tensor.matmul(
                    psa[2][:, 0:NPE2 * NSEG], lhsT=ident,
                    rhs=t84[:, 0:NPE2, :, off], start=(off == 0),
                    stop=(off == 7))
            for slot, qs, n in off_runs:
                if slot >= NPE2:
                    continue
                nc.scalar.copy(
                    S_all[:, NSEG * qs: NSEG * (qs + n)],
                    psa[2][:, NSEG * slot: NSEG * (slot + n)])
            # the last 2 offloaded pairs (g9 tail, contiguous S columns)
            # reduce on a V engine instead
            eng = sch.pick(S2V)
            s2a = pp.tile([128, 2 * NSEG * 4], BF16, tag="s2a", name="s2a")
            s2b = pp.tile([128, 2 * NSEG * 2], BF16, tag="s2b", name="s2b")
            c3 = t8_all[:, NPE2 * NSEG * 8: NOFF * NSEG * 8].rearrange(
                "a (s d) -> a s d", d=8)
            eng.tensor_tensor(
                out=s2a[:].rearrange("a (s d) -> a s d", d=4),
                in0=c3[:, :, 0:4], in1=c3[:, :, 4:8], op=OP.add)
            c3 = s2a[:].rearrange("a (s d) -> a s d", d=4)
            eng.tensor_tensor(
                out=s2b[:].rearrange("a (s d) -> a s d", d=2),
                in0=c3[:, :, 0:2], in1=c3[:, :, 2:4], op=OP.add)
            c3 = s2b[:].rearrange("a (s d) -> a s d", d=2)
            eng.tensor_tensor(
                out=S_all[:, NSEG * 23: NSEG * 25].rearrange(
                    "a (s o) -> a s o", o=1),
                in0=c3[:, :, 0:1], in1=c3[:, :, 1:2], op=OP.add)

            # ---- softmax over the 49 shifts, maxless ----
            # S_b = S + biasS; biasS is host-shifted so S_b <= ~4 (exp-safe)
            # and the unused B half of the single shift p=48 carries -200.
            S_b = pp.tile([128, NQ], F32, tag="Sb", name="Sb")
            eng = sch.pick(_c_tt(NQ, False))
            eng.tensor_tensor(out=S_b[:], in0=S_all[:], in1=bS_s[:],
                              op=OP.add)
            E = pp.tile([128, NQ], BF16, tag="E", name="E")
            nc.scalar.activation(E[:], S_b[:], AF.Exp)
            # per-half denominator sums as a schedulable bf16 add tree
            eng = sch.pick(MTREE)
            dt1 = pp.tile([128, 12 * NSEG], BF16, tag="dt1", name="dt1")
            eng.tensor_tensor(out=dt1[:], in0=E[:, 0:588],
                              in1=E[:, 588:1176], op=OP.add)
            dt2 = pp.tile([128, 6 * NSEG], BF16, tag="dt2", name="dt2")
            eng.tensor_tensor(out=dt2[:], in0=dt1[:, 0:294],
                              in1=dt1[:, 294:588], op=OP.add)
            dt3 = pp.tile([128, 3 * NSEG], BF16, tag="dt3", name="dt3")
            eng.tensor_tensor(out=dt3[:], in0=dt2[:, 0:147],
                              in1=dt2[:, 147:294], op=OP.add)
            dt4 = pp.tile([128, NSEG], BF16, tag="dt4", name="dt4")
            eng.tensor_tensor(out=dt4[:], in0=dt3[:, 0:49],
                              in1=dt3[:, 49:98], op=OP.add)
            dt5 = pp.tile([128, NSEG], BF16, tag="dt5", name="dt5")
            eng.tensor_tensor(out=dt5[:], in0=dt4[:],
                              in1=dt3[:, 98:147], op=OP.add)
            dnq = pp.tile([128, NSEG], BF16, tag="dnq", name="dnq")
            eng.tensor_tensor(out=dnq[:], in0=dt5[:],
                              in1=E[:, 1176:1225], op=OP.add)
            dnb = pp.tile([CH, NSEG], BF16, tag="dnb", name="dnb")
            nc.scalar.copy(dnb[:], dnq[CH:128, :])
            den = pp.tile([CH, NSEG], F32, tag="den", name="den")
            sch.v += (58 + NSEG) / 0.96
            nc.vector.tensor_tensor(out=den[:], in0=dnq[0:CH, :],
                                    in1=dnb[:], op=OP.add)
            rcp = pp.tile([128, NSEG], F32, tag="rcp", name="rcp")
            nc.vector.reciprocal(rcp[0:CH, :], den[:])
            sch.v += (58 + NSEG) / 0.96
            nc.scalar.copy(rcp[CH:128, :], rcp[0:CH, :])
            # pre-normalize the weights so PSUM accumulates the final
            # output directly: the post-AV pass becomes plain ACT evicts
            EN = pp.tile([128, NQ], BF16, tag="EN", name="EN")
            eng = sch.pick(_c_tt(NQ))
            eng.tensor_tensor(
                out=EN[:].rearrange("a (q s) -> a q s", s=NSEG),
                in0=E[:].rearrange("a (q s) -> a q s", s=NSEG),
                in1=rcp[:].rearrange("a (o s) -> a o s", o=1).broadcast_to(
                    (128, NPAIR, NSEG)),
                op=OP.mult)

            # ---- AV: grouped weight-broadcast multiplies + PE half-fold
            # accumulation ----
            first = True
            for g, (q0, G, i, j0, pl) in enumerate(GROUPS):
                vtile = v1 if pl == "k1" else v56
                vp = wp.tile([128, 4 * NPOS], BF16, tag="gp", name="vp",
                             bufs=2)
                ensl = EN[:, NSEG * q0: NSEG * (q0 + G)].rearrange(
                    "a (g s) -> a g s", g=G)
                eng = sch.pick(_c_tt(G * NPOS))
                eng.tensor_tensor(
                    out=vp[:, 0:G * NPOS].rearrange(
                        "a (g s d) -> a g s d", g=G, d=SEG),
                    in0=gwin(vtile, i, j0, G),
                    in1=ensl.unsqueeze(3).broadcast_to(
                        (128, G, NSEG, SEG)),
                    op=OP.mult)
                last = g == len(GROUPS) - 1
                for t in range(G):
                    for kch in range(NAV):
                        nc.tensor.matmul(
                            psa[kch][0:CH, 0:AVC], lhsT=fold,
                            rhs=vp[:, t * NPOS + AVC * kch:
                                   t * NPOS + AVC * (kch + 1)],
                            start=first, stop=(last and t == G - 1))
                    first = False

            # ---- evict the folded PSUM output, store ----
            fin = pp.tile([CH, NPOS], F32, tag="fin", name="fin")
            for kch in range(NAV):
                nc.scalar.copy(fin[:, AVC * kch: AVC * (kch + 1)],
                               psa[kch][0:CH, 0:AVC])
            nc.sync.dma_start(out_d[:], fin[:])
    return nc


import json


def _legalize_waits(bir_bytes):
    """Walrus codegen rejects >1 semaphore wait per instruction; hoist the
    extras onto NoOps (same engine, immediately before) so every
    instruction carries at most one wait."""
    bir = json.loads(bir_bytes)
    ctr = [0]

    def fix_block(instructions):
        out = []
        for ins in instructions:
            si = ins.get("sync_info")
            if si:
                w = si.get("on_wait") or []
                if len(w) > 1:
                    for extra in w[:-1]:
                        ctr[0] += 1
                        out.append({
                            "debug": ins.get("debug", 0),
                            "engine": ins["engine"],
                            "ins": [], "outs": [],
                            "name": f"I-lw{ctr[0]}",
                            "opcode": "NoOp",
                            "sync_info": {"on_wait": [extra],
                                          "on_update": []},
                        })
                    si["on_wait"] = [w[-1]]
            out.append(ins)
        instructions[:] = out

    def walk(o):
        if isinstance(o, dict):
            if "instructions" in o:
                fix_block(o["instructions"])
            for v in o.values():
                walk(v)
        elif isinstance(o, list):
            for v in o:
                walk(v)

    walk(bir)
    return json.dumps(bir).encode()


def _to_bf16_packed(a32):
    """Round fp32 -> bf16 (nearest-even) and pack pairs into fp32 words."""
    u = np.ascontiguousarray(a32, np.float32).view(np.uint32)
    r = ((u + 0x7FFF + ((u >> 16) & 1)) >> 16).astype(np.uint16)
    return np.ascontiguousarray(r).view(np.uint32).view(np.float32)


_NC_CACHE = {}


def kernel(x, q_w, q_b, k_w, k_b, v_w, v_b, h_pos, w_pos):
    x = np.asarray(x, np.float64)
    xp32 = np.pad(x[0], ((0, 0), (3, 3), (3, 3))).reshape(C, NPAD).astype(
        np.float32)
    # pack x for the bf16 conv: partition p holds channels {k*128+p}
    xpb = _to_bf16_packed(
        np.ascontiguousarray(
            xp32.reshape(4, 128, NPAD).transpose(1, 0, 2)).reshape(
            128, 4 * NPAD))
    bias49 = (np.asarray(h_pos, np.float64).sum(0)
              + np.asarray(w_pos, np.float64).sum(0)).reshape(NSH)

    # exact host qsum: sum_d q[c,seg] = q_w[c,:] @ (seg-sums of x) + 64*q_b
    xs = x[0].reshape(C, NSEG, SEG).sum(-1)              # [512, 49] f64
    qsum_all = (np.asarray(q_w, np.float64) @ xs
                + 64.0 * np.asarray(q_b, np.float64)[:, None])  # [512, 49]

    # full positional-bias tensor, shifted per (c,s) so scores stay exp-safe
    # (softmax shift-invariance, exact): biasT[c, s, p] =
    #   qsum[c,s]*bias49[p] - max_p(qsum[c,s]*bias49[p])
    biasT = qsum_all[:, :, None] * bias49[None, None, :]     # [512, 49, 49]
    biasT -= biasT.max(axis=2, keepdims=True)

    eye_u16 = (np.eye(128, dtype=np.uint16) * 0x3F80)    # bf16 1.0
    identp = np.ascontiguousarray(eye_u16.view(np.float32))
    fold_u16 = np.zeros((128, 64), dtype=np.uint16)
    fold_u16[np.arange(128), np.arange(128) % 64] = 0x3F80
    foldp = np.ascontiguousarray(fold_u16.view(np.float32))

    in_maps = []
    chan_lists = []
    for r in range(N_CORES):
        chans = np.array([64 * h + 8 * r + t for h in range(8)
                          for t in range(8)])
        chan_lists.append(chans)
        wq = np.asarray(q_w, np.float32)[chans, :]
        wk = np.asarray(k_w, np.float32)[chans, :]
        wv = np.asarray(v_w, np.float32)[chans, :]
        wTl = np.concatenate([wq.T, wk.T, wv.T], axis=1)     # [512, 192]
        wTb = _to_bf16_packed(
            np.ascontiguousarray(
                wTl.reshape(4, 128, 192).transpose(1, 0, 2)).reshape(
                128, 4 * 192))
        bqkl = np.concatenate([np.asarray(q_b, np.float32)[chans],
                               np.asarray(k_b, np.float32)[chans]])
        # biasS[128, 25*49]: rows 0-63 = A half (shift 2q), 64-127 = B half
        # (shift 2q+1); the dead B half of pair 24 gets -200 (exp -> 0).
        bS = np.empty((128, NPAIR * NSEG), np.float64)
        bt = biasT[chans]                                # [64, 49s, 49p]
        for q in range(NPAIR):
            bS[0:CH, NSEG * q: NSEG * (q + 1)] = bt[:, :, 2 * q]
            if q < NPAIR - 1:
                bS[CH:128, NSEG * q: NSEG * (q + 1)] = bt[:, :, 2 * q + 1]
            else:
                bS[CH:128, NSEG * q: NSEG * (q + 1)] = -200.0
        in_maps.append({
            "xpb": xpb,
            "wTb": wTb,
            "bqk": np.ascontiguousarray(bqkl[:, None]),
            "bv": np.ascontiguousarray(
                np.asarray(v_b, np.float32)[chans][:, None]),
            "bk0": np.ascontiguousarray(
                np.asarray(k_b, np.float32)[chans][:, None]),
            "biasS": np.ascontiguousarray(bS.astype(np.float32)),
            "identp": identp,
            "foldp": foldp,
        })

    if "nc" not in _NC_CACHE:
        nc = _build_nc()
        legal = _legalize_waits(nc.to_json_bytes())
        nc.to_json_bytes = lambda: legal
        _NC_CACHE["nc"] = nc
    res = run_bass_kernel_spmd(_NC_CACHE["nc"], in_maps,
                               list(range(N_CORES)))
    _NC_CACHE["last_results"] = res

    out = np.empty((C, NPOS), np.float32)
    for r in range(N_CORES):
        out[chan_lists[r], :] = np.asarray(res.results[r]["out"])
    return out.reshape(1, C, H, W)


if __name__ == "__main__":
    _build_nc()
    print("build OK")


# revision 22
# speedup vs baseline: 1.0031x; 1.0031x over previous
"""Trainium2 Bass kernel for nn_Attention_layer_12249246728743.

Reference structure (after untangling the C-order reshape): per channel c
of 512, the 3136 raster positions split into 49 segments of 64
consecutive positions; each segment attends over a 7x7 shifted window of
its OWN channel plane (depthwise local attention):

  scores[c,s,p=(i,j)] = sum_d q[c,64s+d] * k[c, win(64s+d, i, j)]
                        + (sum_d q[c,64s+d]) * bias49[p]
  w = softmax_p(scores);  out[c,64s+d] = sum_p w[c,s,p] * v[c, win(...)]

with q/k/v = 1x1 convs of x (k, v on the zero-padded 62x62 domain).

Sharding: channel-parallel across 8 cores (64 channels each); every
attention segment is core-local: no halo, no collectives.

Layout: "pair-packed" attention - partitions = 64 channels x 2
shift-pair halves, free dim = the full 3136-position raster. The B half
holds k/v planes pre-shifted by +1 (or +56 for row-wrapping pairs), so
ONE tensor op computes two of the 49 window shifts at once. Pairs whose
window offsets form an arithmetic run (stride 2 within a row of the 7x7
table) are GROUPED: one 4D strided access pattern computes 3-4 pairs per
tensor op, and the segment-sum bf16 add trees run per group for levels
1-3; level-3 outputs land in shared slot tiles so levels 4/5 run once
across all tree pairs (amortizing per-op overhead). A few pairs offload
their reduction to the PE: stage 1 is 8 strided-rhs identity matmuls
into PSUM 8-sums per pair; stage 2 reduces the first offloaded pairs in
8 shared matmuls while the last two reduce on a vector engine (finest
PE<->vector balance knob). Scores are stored bf16 (the
add trees already round every level to bf16; one more rounding is
noise). The softmax runs WITHOUT a
device max pass: the positional-bias part of the score (qsum x b49,
host-known) is shifted host-side so device scores are bounded above by
~+4; large negatives underflow exp to an exact 0, which softmax
tolerates. AV weights enter the multiply as stride-0 broadcast APs, and
the AV product tiles are accumulated on the PE via half-folding matmuls
(lhsT F[p, p%64]=1) that also combine the two shift halves, so the
output lands directly as [64, 3136] in PSUM. x and the conv weights ship
as bf16 pairs packed in fp32 words (halves the load DMA); the conv runs
in bf16 with fp32 PSUM accumulation. Work is split between DVE and
GPSIMD by an exact subset-sum makespan balancer.
"""

import numpy as np

import concourse.bass as bass
import concourse.mybir as mybir
import concourse.tile as tile
from concourse.bass_utils import run_bass_kernel_spmd
from bass_rust import AP

F32 = mybir.dt.float32
BF16 = mybir.dt.bfloat16
AX = mybir.AxisListType
OP = mybir.AluOpType
AF = mybir.ActivationFunctionType

N_CORES = 8
C = 512
H = W = 56
HP = WP = 62          # padded spatial
NPOS = H * W          # 3136
NPAD = HP * WP        # 3844
K = 7
NSH = K * K           # 49 shifts
SEG = 64              # positions per attention segment
NSEG = NPOS // SEG    # 49 segments per channel
CH = 64               # channels per core
NPAIR = 25            # 24 shift pairs + 1 single (p=48)

# shift pairs (pA=2t, pB=2t+1): B half of the k/v tiles is pre-shifted by
# +1 (same-row j->j+1) or +56 (row wrap (i,6)->(i+1,0)). Window slice is
# always pA's (i,j) clamped to j<=6.
def _pair_table():
    pairs = []
    for t in range(24):
        pA = 2 * t
        i, j = divmod(pA, K)
        if j < K - 1:
            pairs.append((i, j, "k1"))     # B = (i, j+1) via +1 tile
        else:
            pairs.append((i, j, "k56"))    # B = (i+1, 0) via +56 tile
    pairs.append((6, 6, "k1"))             # p=48 single; B half masked
    return pairs


PAIRS = _pair_table()

# groups of pairs whose A-shift offsets are an arithmetic run (stride 2,
# same row i, same source plane): (q0, G, i, j0, plane). One 4D AP
# computes the whole group's products.
GROUPS = [
    (0, 3, 0, 0, "k1"),
    (3, 1, 0, 6, "k56"),
    (4, 3, 1, 1, "k1"),
    (7, 3, 2, 0, "k1"),
    (10, 1, 2, 6, "k56"),
    (11, 3, 3, 1, "k1"),
    (14, 3, 4, 0, "k1"),
    (17, 1, 4, 6, "k56"),
    (18, 3, 5, 1, "k1"),
    (21, 4, 6, 0, "k1"),
]
# PE offload: (group index, #pairs from the group's tail). Tuned so the
# three-engine makespan balances.
OFFLOAD = [(1, 1), (4, 1), (7, 1), (9, 2)]

# score-column permutation: tree-reduced pairs occupy columns 0-19 (in
# chain order) so ONE global op finishes every tree; V-stage2 pairs take
# 20-23 and the PE-stage2 pair takes 24. Groups stay column-contiguous
# (required by the grouped product/AV access patterns).
COLBASE = {0: 0, 1: 24, 2: 3, 3: 6, 4: 22, 5: 9, 6: 12, 7: 23, 8: 15,
           9: 18}
# t8 slot of each offloaded pair (V-stage2 pairs in slots 0-3 matching
# their S columns 20-23; PE-stage2 pair in slot 4 -> column 24)
OFFSLOT = {1: [4], 4: [2], 7: [3], 9: [0, 1]}

# --- naive op-cost mirror (documented TRN2 formulas) for the balancer ---
def _c_tt(fd, bf16_out=True):
    return (58 + (fd / 2 if bf16_out else fd)) / 0.96


class _Sched:
    """DVE/GPSIMD makespan balancer (Pool runs at 2x cost).

    Costs are static, so assignment is planned offline: exact subset-sum
    DP over the schedulable costs against the DVE-pinned base load,
    consumed in emission order by pick().
    """

    def __init__(self, nc, pe_base):
        self.nc = nc
        self.v = 0.0
        self.g = 0.0
        self.pe = pe_base
        self.plan = []

    def make_plan(self, costs, pinned_v, pinned_g=0.0):
        # exact subset-sum DP: choose the GPSIMD subset G minimizing
        # max(pinned_v + S - G, pinned_g + 2G) — globally optimal split
        ic = [int(round(c)) for c in costs]
        total = sum(ic)
        reach = {0: None}
        for idx, c in enumerate(ic):
            new = {}
            for s, _ in reach.items():
                t = s + c
                if t not in reach and t not in new:
                    new[t] = (s, idx)
            reach.update(new)
        pv = int(round(pinned_v))
        pg = int(round(pinned_g))
        best_g = min(reach, key=lambda G: max(pv + total - G, pg + 2 * G))
        gset = set()
        s = best_g
        while s:
            prev, idx = reach[s]
            gset.add(idx)
            s = prev
        asg = ["g" if i in gset else "v" for i in range(len(ic))]
        self.plan = asg[::-1]          # consume via pop()

    def pick(self, cost):
        e = self.plan.pop() if self.plan else (
            "v" if max(self.v + cost, self.g) <= max(self.v,
                                                     self.g + 2 * cost)
            else "g")
        if e == "v":
            self.v += cost
            return self.nc.vector
        self.g += 2 * cost
        return self.nc.gpsimd



def _build_nc():
    nc = bass.Bass()

    # x / conv weights ship as bf16 pairs packed in fp32 words
    xpb = nc.declare_dram_parameter("xpb", [128, 2 * NPAD], F32,
                                    isOutput=False)
    wTb = nc.declare_dram_parameter("wTb", [128, 4 * 96], F32,
                                    isOutput=False)
    bqk = nc.declare_dram_parameter("bqk", [2 * CH, 1], F32, isOutput=False)
    bv = nc.declare_dram_parameter("bv", [CH, 1], F32, isOutput=False)
    bk0 = nc.declare_dram_parameter("bk0", [CH, 1], F32, isOutput=False)
    biasS = nc.declare_dram_parameter("biasS", [128, NPAIR * NSEG], F32,
                                      isOutput=False)
    identp = nc.declare_dram_parameter("identp", [128, 64], F32,
                                       isOutput=False)
    foldp = nc.declare_dram_parameter("foldp", [128, 32], F32,
                                      isOutput=False)
    out_d = nc.declare_dram_parameter("out", [CH, NPOS], F32, isOutput=True)

    RPC = 8                      # rows per conv chunk
    NCHI = RPC * W               # 448 interior positions per chunk
    NCK = 7                      # conv chunks (7*8 = 56 rows)
    AVC = 448                    # AV psum chunk (7 segments)
    NAV = 7                      # AV chunks
    NQ = NPAIR * NSEG            # 1225
    NOFF = sum(n for _, n in OFFLOAD)   # PE-offloaded pair count

    with tile.TileContext(nc) as tc:
        with (
            tc.tile_pool(name="persist", bufs=1) as pp,
            tc.tile_pool(name="work", bufs=2) as wp,
            tc.tile_pool(name="psum", bufs=1, space="PSUM") as psp,
        ):
            MM = lambda n: max(60, 6 + n) / 2.4 + 128 / 1.2
            PE_BASE = (56 * MM(NCHI) + 25 * 7 * MM(448)
                       + NOFF * 8 * MM(392) + 8 * MM((NOFF - 2) * NSEG))
            sch = _Sched(nc, PE_BASE)

            # ---- loads ----
            xt_all = pp.tile([128, 2 * NPAD], F32, tag="xall", name="xall")
            wt_all = pp.tile([128, 4 * 96], F32, tag="wall", name="wall")
            for s0 in range(0, 2 * NPAD, NPAD):
                nc.sync.dma_start(xt_all[:, s0:s0 + NPAD],
                                  xpb[:, s0:s0 + NPAD])
            nc.sync.dma_start(wt_all[:], wTb[:])
            xtb = xt_all[:].bitcast(BF16).rearrange("p (k n) -> p k n", k=4)
            xt = [xtb[:, kt, :] for kt in range(4)]
            wtb = wt_all[:].bitcast(BF16).rearrange("p (k n) -> p k n", k=4)
            wt = [wtb[:, kt, :] for kt in range(4)]
            bqk_s = pp.tile([128, 1], F32, tag="bqk", name="bqk")
            bv_s = pp.tile([CH, 1], F32, tag="bv", name="bv")
            bk0_s = pp.tile([CH, 1], F32, tag="bk0", name="bk0")
            nc.sync.dma_start(bk0_s[:], bk0[:])
            bS_s = pp.tile([128, NQ], F32, tag="bS", name="bS")
            idf_s = pp.tile([128, 64], F32, tag="idf", name="idf")
            fld_s = pp.tile([128, 32], F32, tag="fld", name="fld")
            nc.sync.dma_start(bqk_s[:], bqk[:])
            nc.sync.dma_start(bv_s[:], bv[:])
            nc.sync.dma_start(bS_s[:], biasS[:])
            nc.sync.dma_start(idf_s[:], identp[:])
            nc.sync.dma_start(fld_s[:], foldp[:])
            ident = idf_s[:].bitcast(BF16)          # [128, 128] bf16 identity
            fold = fld_s[:].bitcast(BF16)           # [128, 64] bf16 half-fold

            # ---- attention-layout staging tiles ----
            qb = pp.tile([128, NPOS], BF16, tag="qb", name="qb")
            k1 = pp.tile([128, NPAD], BF16, tag="k1", name="k1")
            k56 = pp.tile([128, NPAD], BF16, tag="k56", name="k56")
            v1 = pp.tile([128, NPAD], BF16, tag="v1", name="v1")
            v56 = pp.tile([128, NPAD], BF16, tag="v56", name="v56")

            # conv / QK-tree / AV shared PSUM accumulators (8 banks)
            psa = [psp.tile([128, 512], F32, tag=f"psa{j}", name=f"psa{j}")
                   for j in range(8)]

            # ---- 1x1 convs on interior rows (rows 3..58 of padded);
            # rhs is a 3D interior view [128, 8, 56] (448 cols) in bf16 --
            for ci in range(NCK):
                r0 = 3 + RPC * ci
                pq = psa[(2 * ci) % 6]
                pv = psa[(2 * ci) % 6 + 1]
                for kt in range(4):
                    x3 = xt[kt].rearrange("a (r c) -> a r c", c=WP)
                    rhs = x3[:, r0:r0 + RPC, 3:59]
                    nc.tensor.matmul(pq[:, 0:NCHI], lhsT=wt[kt][:, 0:128],
                                     rhs=rhs, start=(kt == 0), stop=(kt == 3))
                    nc.tensor.matmul(pv[0:CH, 0:NCHI],
                                     lhsT=wt[kt][:, 128:192],
                                     rhs=rhs, start=(kt == 0), stop=(kt == 3))
                pq3 = pq[:, 0:NCHI].rearrange("a (r c) -> a r c", c=W)
                pv3 = pv[0:CH, 0:NCHI].rearrange("a (r c) -> a r c", c=W)
                qbv = qb[:, NCHI * ci: NCHI * (ci + 1)].rearrange(
                    "a (r c) -> a r c", c=W)
                nc.scalar.activation(qbv[0:CH], pq3[0:CH],
                                     AF.Identity, bias=bqk_s[0:CH, :])
                nc.scalar.activation(qbv[CH:128], pq3[0:CH],
                                     AF.Identity, bias=bqk_s[0:CH, :])
                k13 = k1[0:CH, :].rearrange("a (r c) -> a r c", c=WP)
                v13 = v1[0:CH, :].rearrange("a (r c) -> a r c", c=WP)
                nc.scalar.activation(k13[:, r0:r0 + RPC, 3:59],
                                     pq3[CH:128], AF.Identity,
                                     bias=bqk_s[CH:128, :])
                nc.scalar.activation(v13[:, r0:r0 + RPC, 3:59],
                                     pv3, AF.Identity,
                                     bias=bv_s[:])

            # ---- k/v padded borders hold conv(0)+bias = bias ----
            zt = pp.tile([CH, 1], BF16, tag="zt", name="zt")
            nc.vector.memset(zt[:], 0.0)
            sch.v += (58 + 1) / 0.96
            for plane, bias in ((k1, bk0_s[:]), (v1, bv_s[:])):
                p3 = plane[0:CH, :].rearrange("a (r c) -> a r c", c=WP)
                for view in (
                    plane[0:CH, 0: 3 * WP],                # rows 0-2
                    plane[0:CH, 59 * WP: NPAD],            # rows 59-61
                    p3[:, 3:59, 0:3],                      # left cols
                    p3[:, 3:59, 59:62],                    # right cols
                ):
                    sh = view.shape
                    if len(sh) == 2:
                        zin = zt[:, 0:1].broadcast_to((CH, sh[1]))
                    else:
                        zin = zt[:, 0:1].rearrange(
                            "a (x y) -> a x y", y=1).broadcast_to(
                            (CH, sh[1], sh[2]))
                    nc.scalar.activation(view, zin, AF.Identity, bias=bias)

            # ---- stage shifted B halves (+1 / +56) and k56/v56 A halves --
            for src, d1, d56 in ((k1, k1, k56), (v1, v1, v56)):
                nc.scalar.copy(d1[CH:128, 0:NPAD - 1], src[0:CH, 1:NPAD])
                nc.scalar.copy(d56[0:CH, :], src[0:CH, :])
                nc.scalar.copy(d56[CH:128, 0:NPAD - 56], src[0:CH, 56:NPAD])
                nc.scalar.copy(d1[CH:128, NPAD - 1: NPAD],
                               zt[:, 0:1].broadcast_to((CH, 1)))
                nc.scalar.copy(d56[CH:128, NPAD - 56: NPAD],
                               zt[:, 0:1].broadcast_to((CH, 56)))

            # ---- QK: grouped pair products + bf16 add trees -> scores ----
            S_all = pp.tile([128, NQ], BF16, tag="Sall", name="Sall")
            qb3 = qb[:].rearrange("a (x y) -> a x y", y=W)
            t8_all = pp.tile([128, NOFF * NSEG * 8], BF16, tag="t8a",
                             name="t8a")

            def gwin(t, i, j0, G):
                # [128, G, 56, 56] overlapping window run, stride 2 in j
                base = t[:]
                pstride = base.ap[0][0]
                return AP(base.tensor, WP * i + j0,
                          [[pstride, 128], [2, G], [WP, H], [1, W]])

            offload = {}                 # group idx -> n tail pairs on PE
            for gi, n in OFFLOAD:
                offload[gi] = n

            # tree-chain slot layout: chains write their level-2 outputs
            # into a shared slot tile so levels 3/4/5 AND the final run
            # ONCE across all tree pairs (the column permutation makes
            # the tree pairs' score columns contiguous); 4 offloaded
            # pairs' stage-2 runs as one merged V tree, 1 stays on PE.
            NTT = sum(G - offload.get(g, 0)
                      for g, (_, G, _, _, _) in enumerate(GROUPS))
            # balancer cost table (emission order); tree levels are
            # separate items so the subset-sum split is fine-grained
            COSTS = []
            for g, (q0, G, i, j0, pl) in enumerate(GROUPS):
                COSTS.append(_c_tt(G * NPOS))          # product
                ntree = G - offload.get(g, 0)
                if ntree:
                    COSTS += [_c_tt(ntree * NSEG * w) for w in (32, 16)]
            COSTS += [_c_tt(NTT * NSEG * 8)]           # global level 3
            COSTS += [_c_tt(NTT * NSEG * 4)]           # global level 4
            COSTS += [_c_tt(NTT * NSEG * 2)]           # global level 5
            COSTS += [_c_tt(NTT * NSEG)]               # global final
            S2V = (_c_tt(4 * NSEG * 4) + _c_tt(4 * NSEG * 2)
                   + _c_tt(4 * NSEG))
            COSTS += [S2V]                             # stage-2-on-V pairs
            MTREE = (_c_tt(12 * NSEG) + _c_tt(6 * NSEG) + _c_tt(3 * NSEG)
                     + 3 * _c_tt(NSEG))
            COSTS += [_c_tt(NQ, False)]                # S_b bias add
            COSTS += [MTREE]                           # E denominator tree
            COSTS += [_c_tt(NQ)]                       # EN
            COSTS += [_c_tt(G * NPOS) for (_, G, _, _, _) in GROUPS]  # AV
            PINNED_V = ((58 + 1) / 0.96
                        + (58 + NSEG) / 0.96           # den
                        + (58 + NSEG) / 0.96)          # reciprocal
            # Bass-init emits 4 const-AP memsets on GPSIMD
            PINNED_G = 4 * 2 * (58 + 1) / 0.96
            sch.make_plan(COSTS, PINNED_V, PINNED_G)
            t16a = pp.tile([128, NTT * NSEG * 16], BF16, tag="t16a",
                           name="t16a")
            t8a = pp.tile([128, NTT * NSEG * 8], BF16, tag="t8a2",
                          name="t8a2")
            t4a = pp.tile([128, NTT * NSEG * 4], BF16, tag="t4a",
                          name="t4a")
            t2a = pp.tile([128, NTT * NSEG * 2], BF16, tag="t2a",
                          name="t2a")

            flip = 0
            for g, (q0, G, i, j0, pl) in enumerate(GROUPS):
                ktile = k1 if pl == "k1" else k56
                gp = wp.tile([128, 4 * NPOS], BF16, tag="gp", name="gp",
                             bufs=1)
                eng = sch.pick(_c_tt(G * NPOS))
                eng.tensor_tensor(
                    out=gp[:, 0:G * NPOS].rearrange(
                        "a (g x y) -> a g x y", g=G, y=W),
                    in0=gwin(ktile, i, j0, G),
                    in1=qb3.unsqueeze(1).broadcast_to((128, G, H, W)),
                    op=OP.mult)
                noff = offload.get(g, 0)
                ntree = G - noff
                # PE stage 1 for the group's tail pairs
                for t in range(noff):
                    p3 = gp[:, (ntree + t) * NPOS:
                            (ntree + t + 1) * NPOS].rearrange(
                        "a (s d) -> a s d", d=SEG)
                    pt = psa[flip]
                    flip ^= 1
                    for off in range(0, SEG, 8):
                        nc.tensor.matmul(
                            pt[:, 0:NSEG * 8], lhsT=ident,
                            rhs=p3[:, :, off:off + 8],
                            start=(off == 0), stop=(off == SEG - 8))
                    slot = OFFSLOT[g][t]
                    nc.scalar.copy(
                        t8_all[:, slot * NSEG * 8:
                               (slot + 1) * NSEG * 8].rearrange(
                            "a (s d) -> a s d", d=8),
                        pt[:, 0:NSEG * 8].rearrange("a (s d) -> a s d",
                                                    d=8))
                # DVE tree levels 1-2 over the group's head pairs; level 2
                # lands in the shared slot tile for the global levels
                if ntree:
                    cur = gp[:, 0:ntree * NPOS]
                    slot = COLBASE[g]
                    t = wp.tile([128, 3 * NSEG * 32], BF16,
                                tag="t32", name="t32", bufs=1)
                    c3 = cur.rearrange("a (s d) -> a s d", d=SEG)
                    t_o = t[:, 0:ntree * NSEG * 32]
                    eng = sch.pick(_c_tt(ntree * NSEG * 32))
                    eng.tensor_tensor(
                        out=t_o.rearrange("a (s d) -> a s d", d=32),
                        in0=c3[:, :, 0:32], in1=c3[:, :, 32:64],
                        op=OP.add)
                    c3 = t_o.rearrange("a (s d) -> a s d", d=32)
                    eng = sch.pick(_c_tt(ntree * NSEG * 16))
                    eng.tensor_tensor(
                        out=t16a[:, slot * NSEG * 16:
                                 (slot + ntree) * NSEG * 16].rearrange(
                            "a (s d) -> a s d", d=16),
                        in0=c3[:, :, 0:16], in1=c3[:, :, 16:32],
                        op=OP.add)

            # global tree levels 3/4/5 + final across all tree pairs'
            # slots; the column permutation makes the final's output one
            # contiguous S_all run.
            for src, dst, d in ((t16a, t8a, 16), (t8a, t4a, 8),
                                (t4a, t2a, 4)):
                c3 = src[:].rearrange("a (s d) -> a s d", d=d)
                eng = sch.pick(_c_tt(NTT * NSEG * d // 2))
                eng.tensor_tensor(
                    out=dst[:].rearrange("a (s d) -> a s d", d=d // 2),
                    in0=c3[:, :, 0:d // 2], in1=c3[:, :, d // 2:d],
                    op=OP.add)
            c3 = t2a[:].rearrange("a (s d) -> a s d", d=2)
            eng = sch.pick(_c_tt(NTT * NSEG))
            eng.tensor_tensor(
                out=S_all[:, 0:NTT * NSEG].rearrange(
                    "a (s o) -> a s o", o=1),
                in0=c3[:, :, 0:1], in1=c3[:, :, 1:2], op=OP.add)

            # stage 2: slots 0-3 reduce as one merged V tree into S
            # columns 20-23; slot 4 reduces on the PE into column 24.
            t84 = t8_all[:].rearrange("a (p s d) -> a p s d", s=NSEG, d=8)
            for off in range(8):
                nc.tensor.matmul(
                    psa[2][:, 0:NSEG], lhsT=ident,
                    rhs=t84[:, 4:5, :, off], start=(off == 0),
                    stop=(off == 7))
            nc.scalar.copy(S_all[:, NSEG * 24: NSEG * 25],
                           psa[2][:, 0:NSEG])
            eng = sch.pick(S2V)
            s2a = pp.tile([128, 4 * NSEG * 4], BF16, tag="s2a", name="s2a")
            s2b = pp.tile([128, 4 * NSEG * 2], BF16, tag="s2b", name="s2b")
            c3 = t8_all[:, 0: 4 * NSEG * 8].rearrange(
                "a (s d) -> a s d", d=8)
            eng.tensor_tensor(
                out=s2a[:].rearrange("a (s d) -> a s d", d=4),
                in0=c3[:, :, 0:4], in1=c3[:, :, 4:8], op=OP.add)
            c3 = s2a[:].rearrange("a (s d) -> a s d", d=4)
            eng.tensor_tensor(
                out=s2b[:].rearrange("a (s d) -> a s d", d=2),
                in0=c3[:, :, 0:2], in1=c3[:, :, 2:4], op=OP.add)
            c3 = s2b[:].rearrange("a (s d) -> a s d", d=2)
            eng.tensor_tensor(
                out=S_all[:, NSEG * 20: NSEG * 24].rearrange(
                    "a (s o) -> a s o", o=1),
                in0=c3[:, :, 0:1], in1=c3[:, :, 1:2], op=OP.add)

            # ---- softmax over the 49 shifts, maxless ----
            # S_b = S + biasS; biasS is host-shifted so S_b <= ~4 (exp-safe)
            # and the unused B half of the single shift p=48 carries -200.
            S_b = pp.tile([128, NQ], F32, tag="Sb", name="Sb")
            eng = sch.pick(_c_tt(NQ, False))
            eng.tensor_tensor(out=S_b[:], in0=S_all[:], in1=bS_s[:],
                              op=OP.add)
            E = pp.tile([128, NQ], BF16, tag="E", name="E")
            nc.scalar.activation(E[:], S_b[:], AF.Exp)
            # per-half denominator sums as a schedulable bf16 add tree
            eng = sch.pick(MTREE)
            dt1 = pp.tile([128, 12 * NSEG], BF16, tag="dt1", name="dt1")
            eng.tensor_tensor(out=dt1[:], in0=E[:, 0:588],
                              in1=E[:, 588:1176], op=OP.add)
            dt2 = pp.tile([128, 6 * NSEG], BF16, tag="dt2", name="dt2")
            eng.tensor_tensor(out=dt2[:], in0=dt1[:, 0:294],
                              in1=dt1[:, 294:588], op=OP.add)
            dt3 = pp.tile([128, 3 * NSEG], BF16, tag="dt3", name="dt3")
            eng.tensor_tensor(out=dt3[:], in0=dt2[:, 0:147],
                              in1=dt2[:, 147:294], op=OP.add)
            dt4 = pp.tile([128, NSEG], BF16, tag="dt4", name="dt4")
            eng.tensor_tensor(out=dt4[:], in0=dt3[:, 0:49],
                              in1=dt3[:, 49:98], op=OP.add)
            dt5 = pp.tile([128, NSEG], BF16, tag="dt5", name="dt5")
            eng.tensor_tensor(out=dt5[:], in0=dt4[:],
                              in1=dt3[:, 98:147], op=OP.add)
            dnq = pp.tile([128, NSEG], BF16, tag="dnq", name="dnq")
            eng.tensor_tensor(out=dnq[:], in0=dt5[:],
                              in1=E[:, 1176:1225], op=OP.add)
            dnb = pp.tile([CH, NSEG], BF16, tag="dnb", name="dnb")
            nc.scalar.copy(dnb[:], dnq[CH:128, :])
            den = pp.tile([CH, NSEG], F32, tag="den", name="den")
            sch.v += (58 + NSEG) / 0.96
            nc.vector.tensor_tensor(out=den[:], in0=dnq[0:CH, :],
                                    in1=dnb[:], op=OP.add)
            rcp = pp.tile([128, NSEG], F32, tag="rcp", name="rcp")
            nc.vector.reciprocal(rcp[0:CH, :], den[:])
            sch.v += (58 + NSEG) / 0.96
            nc.scalar.copy(rcp[CH:128, :], rcp[0:CH, :])
            # pre-normalize the weights so PSUM accumulates the final
            # output directly: the post-AV pass becomes plain ACT evicts
            EN = pp.tile([128, NQ], BF16, tag="EN", name="EN")
            eng = sch.pick(_c_tt(NQ))
            eng.tensor_tensor(
                out=EN[:].rearrange("a (q s) -> a q s", s=NSEG),
                in0=E[:].rearrange("a (q s) -> a q s", s=NSEG),
                in1=rcp[:].rearrange("a (o s) -> a o s", o=1).broadcast_to(
                    (128, NPAIR, NSEG)),
                op=OP.mult)

            # ---- AV: grouped weight-broadcast multiplies + PE half-fold
            # accumulation ----
            first = True
            for g, (q0, G, i, j0, pl) in enumerate(GROUPS):
                vtile = v1 if pl == "k1" else v56
                vp = wp.tile([128, 4 * NPOS], BF16, tag="gp", name="vp",
                             bufs=1)
                cb = COLBASE[g]
                ensl = EN[:, NSEG * cb: NSEG * (cb + G)].rearrange(
                    "a (g s) -> a g s", g=G)
                eng = sch.pick(_c_tt(G * NPOS))
                eng.tensor_tensor(
                    out=vp[:, 0:G * NPOS].rearrange(
                        "a (g s d) -> a g s d", g=G, d=SEG),
                    in0=gwin(vtile, i, j0, G),
                    in1=ensl.unsqueeze(3).broadcast_to(
                        (128, G, NSEG, SEG)),
                    op=OP.mult)
                last = g == len(GROUPS) - 1
                for t in range(G):
                    for kch in range(NAV):
                        nc.tensor.matmul(
                            psa[kch][0:CH, 0:AVC], lhsT=fold,
                            rhs=vp[:, t * NPOS + AVC * kch:
                                   t * NPOS + AVC * (kch + 1)],
                            start=first, stop=(last and t == G - 1))
                    first = False

            # ---- evict the folded PSUM output, store ----
            fin = pp.tile([CH, NPOS], F32, tag="fin", name="fin")
            for kch in range(NAV):
                nc.scalar.copy(fin[:, AVC * kch: AVC * (kch + 1)],
                               psa[kch][0:CH, 0:AVC])
            nc.sync.dma_start(out_d[:], fin[:])
    return nc


import json


def _legalize_waits(bir_bytes):
    """Walrus codegen rejects >1 semaphore wait per instruction; hoist the
    extras onto NoOps (same engine, immediately before) so every
    instruction carries at most one wait."""
    bir = json.loads(bir_bytes)
    ctr = [0]

    def fix_block(instructions):
        out = []
        for ins in instructions:
            si = ins.get("sync_info")
            if si:
                w = si.get("on_wait") or []
                if len(w) > 1:
                    for extra in w[:-1]:
                        ctr[0] += 1
                        out.append({
                            "debug": ins.get("debug", 0),
                            "engine": ins["engine"],
                            "ins": [], "outs": [],
                            "name": f"I-lw{ctr[0]}",
                            "opcode": "NoOp",
                            "sync_info": {"on_wait": [extra],
                                          "on_update": []},
                        })
                    si["on_wait"] = [w[-1]]
            out.append(ins)
        instructions[:] = out

    def walk(o):
        if isinstance(o, dict):
            if "instructions" in o:
                fix_block(o["instructions"])
            for v in o.values():
                walk(v)
        elif isinstance(o, list):
            for v in o:
                walk(v)

    walk(bir)
    return json.dumps(bir).encode()


def _to_bf16_packed(a32):
    """Round fp32 -> bf16 (nearest-even) and pack pairs into fp32 words."""
    u = np.ascontiguousarray(a32, np.float32).view(np.uint32)
    r = ((u + 0x7FFF + ((u >> 16) & 1)) >> 16).astype(np.uint16)
    return np.ascontiguousarray(r).view(np.uint32).view(np.float32)


_NC_CACHE = {}


def kernel(x, q_w, q_b, k_w, k_b, v_w, v_b, h_pos, w_pos):
    x = np.asarray(x, np.float64)
    xp32 = np.pad(x[0], ((0, 0), (3, 3), (3, 3))).reshape(C, NPAD).astype(
        np.float32)
    # pack x for the bf16 conv: partition p holds channels {k*128+p}
    xpb = _to_bf16_packed(
        np.ascontiguousarray(
            xp32.reshape(4, 128, NPAD).transpose(1, 0, 2)).reshape(
            128, 4 * NPAD))
    bias49 = (np.asarray(h_pos, np.float64).sum(0)
              + np.asarray(w_pos, np.float64).sum(0)).reshape(NSH)

    # exact host qsum: sum_d q[c,seg] = q_w[c,:] @ (seg-sums of x) + 64*q_b
    xs = x[0].reshape(C, NSEG, SEG).sum(-1)              # [512, 49] f64
    qsum_all = (np.asarray(q_w, np.float64) @ xs
                + 64.0 * np.asarray(q_b, np.float64)[:, None])  # [512, 49]

    # full positional-bias tensor, shifted per (c,s) so scores stay exp-safe
    # (softmax shift-invariance, exact): biasT[c, s, p] =
    #   qsum[c,s]*bias49[p] - max_p(qsum[c,s]*bias49[p])
    biasT = qsum_all[:, :, None] * bias49[None, None, :]     # [512, 49, 49]
    biasT -= biasT.max(axis=2, keepdims=True)

    eye_u16 = (np.eye(128, dtype=np.uint16) * 0x3F80)    # bf16 1.0
    identp = np.ascontiguousarray(eye_u16.view(np.float32))
    fold_u16 = np.zeros((128, 64), dtype=np.uint16)
    fold_u16[np.arange(128), np.arange(128) % 64] = 0x3F80
    foldp = np.ascontiguousarray(fold_u16.view(np.float32))

    in_maps = []
    chan_lists = []
    for r in range(N_CORES):
        chans = np.array([64 * h + 8 * r + t for h in range(8)
                          for t in range(8)])
        chan_lists.append(chans)
        wq = np.asarray(q_w, np.float32)[chans, :]
        wk = np.asarray(k_w, np.float32)[chans, :]
        wv = np.asarray(v_w, np.float32)[chans, :]
        wTl = np.concatenate([wq.T, wk.T, wv.T], axis=1)     # [512, 192]
        wTb = _to_bf16_packed(
            np.ascontiguousarray(
                wTl.reshape(4, 128, 192).transpose(1, 0, 2)).reshape(
                128, 4 * 192))
        bqkl = np.concatenate([np.asarray(q_b, np.float32)[chans],
                               np.asarray(k_b, np.float32)[chans]])
        # biasS[128, 25*49] in PERMUTED column order (COLBASE): rows 0-63
        # = A half (shift 2q), 64-127 = B half (shift 2q+1); the dead B
        # half of pair 24 gets -200 (exp -> 0).
        bS = np.empty((128, NPAIR * NSEG), np.float64)
        bt = biasT[chans]                                # [64, 49s, 49p]
        for g, (q0, G, _, _, _) in enumerate(GROUPS):
            for t in range(G):
                q = q0 + t
                col = COLBASE[g] + t
                bS[0:CH, NSEG * col: NSEG * (col + 1)] = bt[:, :, 2 * q]
                if q < NPAIR - 1:
                    bS[CH:128, NSEG * col: NSEG * (col + 1)] = \
                        bt[:, :, 2 * q + 1]
                else:
                    bS[CH:128, NSEG * col: NSEG * (col + 1)] = -200.0
        in_maps.append({
            "xpb": xpb,
            "wTb": wTb,
            "bqk": np.ascontiguousarray(bqkl[:, None]),
            "bv": np.ascontiguousarray(
                np.asarray(v_b, np.float32)[chans][:, None]),
            "bk0": np.ascontiguousarray(
                np.asarray(k_b, np.float32)[chans][:, None]),
            "biasS": np.ascontiguousarray(bS.astype(np.float32)),
            "identp": identp,
            "foldp": foldp,
        })

    if "nc" not in _NC_CACHE:
        nc = _build_nc()
        legal = _legalize_waits(nc.to_json_bytes())
        nc.to_json_bytes = lambda: legal
        _NC_CACHE["nc"] = nc
    res = run_bass_kernel_spmd(_NC_CACHE["nc"], in_maps,
                               list(range(N_CORES)))
    _NC_CACHE["last_results"] = res

    out = np.empty((C, NPOS), np.float32)
    for r in range(N_CORES):
        out[chan_lists[r], :] = np.asarray(res.results[r]["out"])
    return out.reshape(1, C, H, W)


if __name__ == "__main__":
    _build_nc()
    print("build OK")
